# revision 10
# baseline (speedup 1.0000x reference)
"""Trainium2 Bass kernel for nn_AMMaskedLinear (v2).

Math: the reference's per-sample weight mask is separable:
    weight_mask[b,o,i] = pl[b,i] * ph[b,o] * S[o,i]
with
    present[b,v] = any_j(hidden_rank[b,j] == v)            (v in 0..32)
    pl[b,i] = present[b, r_low[i]]  & (r_low[i]  != 0)
    om[b,o] = present[b, r_high[o]]
    S&nz[o,i] = (r_low[i] <= r_high[o]) & (r_high[o] != 0)
              = (r_high[o] >= max(r_low[i], 1))
so with E[j,k] = S&nz[j,k] * direction^T[j,k]:
    Y[k,b]   = sum_j E[j,k] * (pl[j,b]*x[j,b])
    out[k,b] = om[k,b] * (cscale_b[k] * Y[k,b] + cbias_b[k])

Presence via bit-pack on a [128=(half,b), 512] layout (u16 words; the
neuronxcc ISA check rejects 32-bit DVE shifts):
  w0 = 1 << min(hr, 16)        bits 1..15 <-> values 1..15
  w1 = 1 << max(hr - 15, 0)    bits 1..15 <-> values 16..30
  (all clamp-boundary bits land on bit 0 or shift out entirely, so they are
  dead under either mod-16 or saturating shift semantics)
  flags (GpSimd): f0 = (hr == 31), f1 = (hr >= 32), f2 = (hr <= 0)
  OR-reduce along the free axis (one tensor_reduce per word group), OR the
  two row halves, then extract 33 presence columns:
    p33[:, c]     = w0 & (2 << c)         c = 0..14  -> value c+1,  scale 2^(c+1)
    p33[:, c]     = w1 & (2 << (c - 15))  c = 15..29 -> value c+1,  scale 2^(c-14)
    p33[:, 30:33] = flags                 values 31, 32, 0          ({0,1})
  The power-of-two scale is compensated by a 2^-bit factor baked into the
  one-hot gather tables (exact in bf16).

Distribution: OUT sharded over 8 cores (direction is the only big tensor).
Engines: DVE owns the pack chain + final muls; GpSimd computes the flag
words, the S&nz mask and E; PE transposes presence and does the gathers +
main matmul; Scalar only issues bulk DMAs; Sync issues hr/aux DMAs.
"""

import numpy as np

B, IN, OUT, D = 64, 1024, 1024, 32
NCORES = 8
KSH = OUT // NCORES  # 128 outputs per core
NT = IN // 128       # 8 contraction tiles

# aux2 [128, A2W] f32
A_RLPP = 0           # [128, 8]    r_low partition-major f32
A_RHBB = 8           # [128, 128]  r_high shard bcast f32
A_CS = 136           # [128, 1]    cscale_b shard (per-partition k)
A_CB = 137           # [128, 1]    cbias_b shard
A_MSK = 138          # [64, 30]    u16 masks 2<<c for w0/w1 (15 f32 cols)
A_IDENT = 153        # [64, 64]    bf16 identity (32 f32 cols)
A_VPL = 185          # [33, 1]     viota for pl one-hot (v0 row dead)
A_VOM = 186          # [33, 1]     viota for om one-hot
A_POW = 187          # [33, 1]     2^-(bit) compensation factors
A2W = 188

# v33 [33, VW33] f32 (bf16 payload)
V_RL = 0             # [33, 1024] bf16 r_low bcast
V_RH = 1024          # [33, 128]  bf16 r_high shard bcast
VW33 = 576           # f32 cols = 1152 bf16

HRW = 256            # hrp [128, 256] f32 = [128, 512] i16
DIRW = 512           # dirT [128, 512] f32 = [128, 8, 128] bf16
XW = 256             # xTp  [128, 256] f32 = [128, 8, 64] bf16

_cached = {}


def _build_nc():
    import contextlib

    import concourse.bass as bass
    import concourse.mybir as mybir

    f32 = mybir.dt.float32
    bf16 = mybir.dt.bfloat16
    i16 = mybir.dt.int16
    u16 = mybir.dt.uint16
    u32 = mybir.dt.uint32
    Alu = mybir.AluOpType

    nc = bass.Bass()

    hrp_h = nc.declare_dram_parameter("hrp", [128, HRW], f32, isOutput=False)
    aux2_h = nc.declare_dram_parameter("aux2", [128, A2W], f32, isOutput=False)
    v33_h = nc.declare_dram_parameter("v33", [33, VW33], f32, isOutput=False)
    dirT_h = nc.declare_dram_parameter("dirT", [128, DIRW], f32, isOutput=False)
    xTp_h = nc.declare_dram_parameter("xTp", [128, XW], f32, isOutput=False)
    out_h = nc.declare_dram_parameter("out", [KSH, B], f32, isOutput=True)

    ctx = contextlib.ExitStack()

    def sb(name, shape, dt=f32):
        return ctx.enter_context(nc.sbuf_tensor(name, shape, dt))[:]

    def ps(name, shape, dt=f32):
        return ctx.enter_context(nc.psum_tensor(name, shape, dt))[:]

    with ctx:
        hrp_t = sb("hrp_t", [128, HRW])
        aux2_t = sb("aux2_t", [128, A2W])
        v33_t = sb("v33_t", [33, VW33])
        dirT_t = sb("dirT_t", [128, DIRW])
        xTp_t = sb("xTp_t", [128, XW])

        ones_t = sb("ones_t", [128, 2, 512], u16)
        amt_t = sb("amt_t", [128, 2, 512], u16)
        w01_t = sb("w01_t", [128, 2, 512], u16)
        wf_t = sb("wf_t", [128, 3, 512], u16)
        packed_t = sb("packed_t", [128, 5], u16)   # 0:2 = w0/w1, 2:5 = flags
        packhi_t = sb("packhi_t", [64, 5], u16)
        p33u_t = sb("p33u_t", [64, 33], u16)
        p33_t = sb("p33_t", [64, 33], bf16)
        ident_t = sb("ident_t", [64, 64], bf16)
        presT_t = sb("presT_t", [33, 64], bf16)
        ohlow_t = sb("ohlow_t", [33, IN], bf16)
        ohhigh_t = sb("ohhigh_t", [33, KSH], bf16)
        rlmax_t = sb("rlmax_t", [128, NT])
        mask_t = sb("mask_t", [128, NT, KSH], bf16)
        E_t = sb("E_t", [128, NT, KSH], bf16)
        xlT_t = sb("xlT_t", [128, NT, B], bf16)
        z_t = sb("z_t", [KSH, B])
        outT_t = sb("outT_t", [KSH, B])

        presT_ps = ps("presT_ps", [33, 64], bf16)
        plT_ps = ps("plT_ps", [128, NT, B])
        omT_ps = ps("omT_ps", [KSH, B])
        Y_ps = ps("Y_ps", [KSH, B])

        hr_ap = hrp_t[:, :].bitcast(i16)                        # [128, 512]
        rlowpp_ap = aux2_t[:, A_RLPP : A_RLPP + NT]
        rhighbb_ap = aux2_t[:, A_RHBB : A_RHBB + KSH]
        cs_ap = aux2_t[:, A_CS : A_CS + 1]
        cb_ap = aux2_t[:, A_CB : A_CB + 1]
        msk_ap = aux2_t[0:64, A_MSK : A_MSK + 15].bitcast(u16)  # [64, 30]
        ident_ap = aux2_t[0:64, A_IDENT : A_IDENT + 32].bitcast(bf16)
        vpl_ap = aux2_t[0:33, A_VPL : A_VPL + 1]
        vom_ap = aux2_t[0:33, A_VOM : A_VOM + 1]
        pow_ap = aux2_t[0:33, A_POW : A_POW + 1]
        rl33_ap = v33_t[:, V_RL // 2 : (V_RL + IN) // 2].bitcast(bf16)
        rh33_ap = v33_t[:, V_RH // 2 : (V_RH + KSH) // 2].bitcast(bf16)
        xT_ap = xTp_t[:, :].bitcast(bf16).rearrange("p (t b) -> p t b", t=NT)
        dirT_ap = dirT_t[:, :].bitcast(bf16).rearrange("p (t k) -> p t k", t=NT)

        hr_sem = ctx.enter_context(nc.semaphore("hr_sem"))
        aux_sem = ctx.enter_context(nc.semaphore("aux_sem"))
        dir_sem = ctx.enter_context(nc.semaphore("dir_sem"))
        x_sem = ctx.enter_context(nc.semaphore("x_sem"))
        out_sem = ctx.enter_context(nc.semaphore("out_sem"))
        dve_sem = ctx.enter_context(nc.semaphore("dve_sem"))
        pe_sem = ctx.enter_context(nc.semaphore("pe_sem"))
        gp_sem = ctx.enter_context(nc.semaphore("gp_sem"))
        block = ctx.enter_context(nc.Block())

        @block.sync
        def _(sync):
            sync.dma_start(out=hrp_t, in_=hrp_h[:, :]).then_inc(hr_sem, 16)
            sync.dma_start(out=aux2_t, in_=aux2_h[:, :]).then_inc(aux_sem, 16)
            sync.wait_ge(out_sem, 16)

        @block.scalar
        def _(scalar):
            scalar.dma_start(out=v33_t, in_=v33_h[:, :]).then_inc(aux_sem, 16)
            scalar.dma_start(out=dirT_t, in_=dirT_h[:, :]).then_inc(dir_sem, 16)
            scalar.dma_start(out=xTp_t, in_=xTp_h[:, :]).then_inc(x_sem, 16)
            scalar.wait_ge(dve_sem, 4)
            scalar.dma_start(out=out_h[:, :], in_=outT_t).then_inc(out_sem, 16)

        @block.gpsimd
        def _(gpsimd):
            gpsimd.wait_ge(hr_sem, 16)
            # {0,1} flag words for values outside the two u16 bit windows
            nc.gpsimd.tensor_scalar(
                out=wf_t[:, 0, :], in0=hr_ap, scalar1=31, scalar2=None,
                op0=Alu.is_equal,
            )
            nc.gpsimd.tensor_scalar(
                out=wf_t[:, 1, :], in0=hr_ap, scalar1=32, scalar2=None,
                op0=Alu.is_ge,
            )
            nc.gpsimd.tensor_scalar(
                out=wf_t[:, 2, :], in0=hr_ap, scalar1=0, scalar2=None,
                op0=Alu.is_le,
            ).then_inc(gp_sem, 1)
            # gp=1: flag words ready (DVE reduces them)
            gpsimd.wait_ge(aux_sem, 32)
            nc.gpsimd.tensor_scalar(
                out=rlmax_t, in0=rlowpp_ap, scalar1=1.0, scalar2=None,
                op0=Alu.max,
            )
            gpsimd.drain()
            # S&nz mask: mask[p,t,k] = (r_high[k] >= max(r_low[t*128+p], 1))
            for t in range(NT):
                nc.gpsimd.tensor_scalar(
                    out=mask_t[:, t, :], in0=rhighbb_ap,
                    scalar1=rlmax_t[:, t : t + 1], scalar2=None, op0=Alu.is_ge,
                )
            gpsimd.drain()
            gpsimd.wait_ge(dir_sem, 16)
            nc.gpsimd.tensor_mul(out=E_t, in0=mask_t, in1=dirT_ap).then_inc(
                gp_sem, 1
            )
            # gp=2: E ready

        @block.vector
        def _(vector):
            nc.vector.memset(ones_t, 1)
            vector.drain()
            vector.wait_ge(hr_sem, 16)
            # shift amounts: w0 window min(hr,16); w1 window max(hr-15,0)
            nc.vector.tensor_scalar(
                out=amt_t[:, 0, :], in0=hr_ap, scalar1=16, scalar2=None,
                op0=Alu.min,
            )
            nc.vector.tensor_scalar(
                out=amt_t[:, 1, :], in0=hr_ap, scalar1=15, scalar2=0,
                op0=Alu.subtract, op1=Alu.max,
            )
            vector.drain()
            nc.vector.tensor_tensor(
                out=w01_t, in0=ones_t, in1=amt_t, op=Alu.logical_shift_left,
            )
            vector.drain()
            nc.vector.tensor_reduce(
                out=packed_t[:, 0:2], in_=w01_t,
                axis=mybir.AxisListType.X, op=Alu.bitwise_or,
            )
            vector.wait_ge(gp_sem, 1)
            nc.vector.tensor_reduce(
                out=packed_t[:, 2:5], in_=wf_t,
                axis=mybir.AxisListType.X, op=Alu.bitwise_or,
            )
            # one-hot gather tables ride the drain window (v33 landed long ago)
            vector.wait_ge(aux_sem, 32)
            nc.vector.tensor_scalar(
                out=ohlow_t, in0=rl33_ap, scalar1=vpl_ap, scalar2=pow_ap,
                op0=Alu.is_equal, op1=Alu.mult,
            )
            nc.vector.tensor_scalar(
                out=ohhigh_t, in0=rh33_ap, scalar1=vom_ap, scalar2=pow_ap,
                op0=Alu.is_equal, op1=Alu.mult,
            )
            nc.vector.tensor_copy(out=ident_t, in_=ident_ap)
            vector.drain()
            # combine the two row-halves (upper half staged through a copy)
            nc.vector.tensor_copy(out=packhi_t, in_=packed_t[64:128, :])
            vector.drain()
            nc.vector.tensor_tensor(
                out=packed_t[0:64, :], in0=packed_t[0:64, :], in1=packhi_t,
                op=Alu.bitwise_or,
            )
            vector.drain()
            # extract 33 presence columns (values scaled 2^bit; compensated
            # in the one-hot tables); bitwise ops must stay integer-typed,
            # the u16->bf16 numeric convert rides the final copy
            nc.vector.tensor_tensor(
                out=p33u_t[:, 0:15],
                in0=packed_t[0:64, 0:1].broadcast_to((64, 15)),
                in1=msk_ap[:, 0:15], op=Alu.bitwise_and,
            )
            nc.vector.tensor_tensor(
                out=p33u_t[:, 15:30],
                in0=packed_t[0:64, 1:2].broadcast_to((64, 15)),
                in1=msk_ap[:, 15:30], op=Alu.bitwise_and,
            )
            nc.vector.tensor_copy(out=p33u_t[:, 30:33], in_=packed_t[0:64, 2:5])
            vector.drain()
            nc.vector.tensor_copy(out=p33_t, in_=p33u_t).then_inc(dve_sem, 1)
            # dve=1: p33 + ident + one-hots ready
            vector.wait_ge(pe_sem, 1)
            nc.vector.tensor_copy(out=presT_t, in_=presT_ps).then_inc(dve_sem, 1)
            # dve=2: presT in SBUF
            vector.wait_ge(x_sem, 16)
            vector.wait_ge(pe_sem, 2)
            nc.vector.tensor_mul(out=xlT_t, in0=xT_ap, in1=plT_ps).then_inc(
                dve_sem, 1
            )
            # dve=3: xlT ready
            vector.wait_ge(pe_sem, 3)
            nc.vector.scalar_tensor_tensor(
                out=z_t, in0=Y_ps, scalar=cs_ap,
                in1=cb_ap.broadcast_to((KSH, B)),
                op0=Alu.mult, op1=Alu.add,
            )
            vector.drain()
            nc.vector.tensor_mul(out=outT_t, in0=omT_ps, in1=z_t).then_inc(
                dve_sem, 1
            )
            # dve=4: outT ready (scalar issues the store)

        @block.tensor
        def _(tensor):
            tensor.wait_ge(dve_sem, 1)
            nc.tensor.transpose(presT_ps, p33_t, ident_t).then_inc(pe_sem, 1)
            # pe=1: presT_ps ready
            tensor.wait_ge(dve_sem, 2)
            for t in range(NT):
                ins = nc.tensor.matmul(
                    plT_ps[:, t, :],
                    ohlow_t[0:33, t * 128 : (t + 1) * 128],
                    presT_t[0:33, :],
                )
            ins.then_inc(pe_sem, 1)
            # pe=2: plT ready
            nc.tensor.matmul(omT_ps, ohhigh_t[0:33, :], presT_t[0:33, :])
            tensor.wait_ge(dve_sem, 3)
            tensor.wait_ge(gp_sem, 2)
            for t in range(NT):
                ins = nc.tensor.matmul(
                    Y_ps, E_t[:, t, :], xlT_t[:, t, :],
                    start=(t == 0), stop=(t == NT - 1),
                )
            ins.then_inc(pe_sem, 1)
            # pe=3: Y ready

    return nc


def _host_tables():
    import ml_dtypes

    ident = np.eye(64, dtype=ml_dtypes.bfloat16)
    bits = np.concatenate([np.arange(1, 16), np.arange(1, 16)])  # per c 0..29
    masks = (np.uint16(1) << bits.astype(np.uint16))[None, :].repeat(64, 0)
    viota_pl = np.empty((33, 1), np.float32)
    viota_pl[0:30, 0] = np.arange(1, 31)
    viota_pl[30, 0] = 31.0
    viota_pl[31, 0] = 32.0
    viota_pl[32, 0] = -1.0
    viota_om = viota_pl.copy()
    viota_om[32, 0] = 0.0
    powv = np.empty((33, 1), np.float32)
    powv[0:30, 0] = 2.0 ** -bits[0:30]
    powv[30, 0] = 1.0
    powv[31, 0] = 1.0
    powv[32, 0] = 1.0
    return ident, masks, viota_pl, viota_om, powv


def _prep_in_maps(inputs):
    """Host-side sharding: layout / dtype transforms only, no arithmetic."""
    import ml_dtypes

    bf = ml_dtypes.bfloat16
    x = np.ascontiguousarray(np.asarray(inputs["x"], dtype=np.float32))
    hr = np.ascontiguousarray(np.asarray(inputs["hidden_rank"], dtype=np.int32))
    r_low = np.asarray(inputs["r_low"], dtype=np.int32)
    r_high = np.asarray(inputs["r_high"], dtype=np.int32)
    direction = np.asarray(inputs["direction"], dtype=np.float32)
    cscale_b = np.asarray(inputs["cscale_b"], dtype=np.float32)
    cbias_b = np.asarray(inputs["cbias_b"], dtype=np.float32)

    # partition p = h*64 + b, free = s: hr2[h*64+b, s] = hr[b, h*512+s]
    hr2 = hr.reshape(B, 2, 512).transpose(1, 0, 2).reshape(128, 512)
    hrp = hr2.astype(np.int16).view(np.float32)  # [128, 256]

    xT3 = x.T.reshape(NT, 128, B).transpose(1, 0, 2)  # [128, NT, B]
    xTp = xT3.reshape(128, -1).astype(bf).view(np.float32)

    rlowf = r_low.astype(np.float32)
    rhighf = r_high.astype(np.float32)
    ident, masks, viota_pl, viota_om, powv = _host_tables()

    aux2 = np.zeros((128, A2W), np.float32)
    aux2[:, A_RLPP : A_RLPP + NT] = rlowf.reshape(NT, 128).T
    aux2[0:64, A_MSK : A_MSK + 15] = masks.view(np.float32)
    aux2[0:64, A_IDENT : A_IDENT + 32] = ident.view(np.float32)
    aux2[0:33, A_VPL : A_VPL + 1] = viota_pl
    aux2[0:33, A_VOM : A_VOM + 1] = viota_om
    aux2[0:33, A_POW : A_POW + 1] = powv

    v33 = np.zeros((33, VW33 * 2), bf)
    v33[:, V_RL : V_RL + IN] = rlowf[None, :].astype(bf)

    in_maps = []
    for c in range(NCORES):
        sl = slice(c * KSH, (c + 1) * KSH)
        rh = rhighf[sl]
        aux2c = aux2.copy()
        aux2c[:, A_RHBB : A_RHBB + KSH] = rh[None, :]
        aux2c[:, A_CS] = cscale_b[sl]
        aux2c[:, A_CB] = cbias_b[sl]
        v33c = v33.copy()
        v33c[:, V_RH : V_RH + KSH] = rh[None, :].astype(bf)
        dirT = (
            direction[sl, :].T.reshape(NT, 128, KSH).transpose(1, 0, 2)
            .reshape(128, -1).astype(bf).view(np.float32)
        )
        in_maps.append(
            {
                "hrp": hrp,
                "aux2": aux2c,
                "v33": v33c.view(np.float32),
                "dirT": dirT,
                "xTp": xTp,
            }
        )
    return in_maps


def _run(inputs, trace=False, **kw):
    from concourse.bass_utils import run_bass_kernel_spmd

    if "nc" not in _cached:
        _cached["nc"] = _build_nc()
    nc = _cached["nc"]
    in_maps = _prep_in_maps(inputs)
    res = run_bass_kernel_spmd(
        nc, in_maps, core_ids=list(range(NCORES)), trace=trace, **kw
    )
    out = np.concatenate([np.asarray(r["out"]).T for r in res.results], axis=1)
    return out.astype(np.float32), res


def kernel(**inputs):
    out, _ = _run(inputs, trace=False)
    return out


# revision 12
# speedup vs baseline: 2.0891x; 2.0891x over previous
"""Trainium2 Bass kernel for nn_AMMaskedLinear (v3).

Math: the reference's per-sample weight mask is separable:
    present[b,v] = any_j(hidden_rank[b,j] == v)            (v in 0..32)
    pl[b,i] = present[b, r_low[i]]  & (r_low[i]  != 0)
    om[b,o] = present[b, r_high[o]]
    E[j,k]  = (r_high[k] >= max(r_low[j], 1)) * direction^T[j,k]
    Y[k,b]  = sum_j E[j,k] * pl[j,b] * x[j,b]
    out[k,b] = om[k,b] * (cscale_b[k] * Y[k,b] + cbias_b[k])

Presence bit-pack on [128=(half,b), 512] (3 u16 words, baseline-proven
clamp windows; all clamp-boundary bits are dead):
    w0 = 1 << min(hr, 15)              bits 0..14 <-> values 0..14
    w1 = 1 << clamp(hr-14, 0, 15)      bits 1..14 <-> values 15..28
    w2 = 1 << max(hr-28, 0)            bits 1..4  <-> values 29..32
One tensor_reduce(bitwise_or) along the free axis replaces the baseline's
9-level OR tree.  Extraction is 3 bitwise_ANDs with per-column masks; the
resulting power-of-two scales are compensated by 2^-bit factors baked into
the one-hot gather tables (exact in bf16), so no booleanize pass is needed.

Engine split: DVE owns pack + one-hots + E + xlT + final mul; PE transposes
presence and runs gathers + main matmul; ACT copies presT out of PSUM and
computes z = cs*Y + cb (per-partition scale/bias APs) and issues the bulk
DMAs; Sync issues hr/aux2.  GpSimd is UNUSED on purpose: measured ~27x
slower than DVE per element and it stalls concurrent DVE ops.
"""

import numpy as np

B, IN, OUT, D = 64, 1024, 1024, 32
NCORES = 8
KSH = OUT // NCORES  # 128 outputs per core
NT = IN // 128       # 8 contraction tiles

# aux2 [128, A2W] f32
A_RLPP = 0           # [128, 4]    r_low partition-major bf16 (8 bf16)
A_RHBB = 4           # [128, 64]   r_high shard bcast bf16 (128 bf16)
A_CS = 68            # [128, 1]    cscale_b shard f32 (per-partition k)
A_CB = 69            # [128, 1]    cbias_b shard f32
A_MSK = 70           # [64, 17]    u16 extraction masks (33 used + pad)
A_VPL = 87           # [33, 1]     viota for pl one-hot (v0 col dead)
A_VOM = 88           # [33, 1]     viota for om one-hot
A_POW = 89           # [33, 1]     2^-bit compensation factors
A2W = 90

# v33 [33, VW33] f32 (bf16 payload)
V_RL = 0             # [33, 1024] bf16 r_low bcast
V_RH = 1024          # [33, 128]  bf16 r_high shard bcast
VW33 = 576

HRW = 256            # hrp [128, 256] f32 = [128, 512] i16
DIRW = 512           # dirT [128, 512] f32 = [128, 8, 128] bf16
XW = 256             # xTp  [128, 256] f32 = [128, 8, 64] bf16

_cached = {}


def _build_nc():
    import contextlib

    import concourse.bass as bass
    import concourse.mybir as mybir

    f32 = mybir.dt.float32
    bf16 = mybir.dt.bfloat16
    i16 = mybir.dt.int16
    u16 = mybir.dt.uint16
    Alu = mybir.AluOpType
    Act = mybir.ActivationFunctionType

    nc = bass.Bass()

    hrp_h = nc.declare_dram_parameter("hrp", [128, HRW], f32, isOutput=False)
    aux2_h = nc.declare_dram_parameter("aux2", [128, A2W], f32, isOutput=False)
    identp_h = nc.declare_dram_parameter("identp", [64, 64], bf16, isOutput=False)
    v33_h = nc.declare_dram_parameter("v33", [33, VW33], f32, isOutput=False)
    dirT_h = nc.declare_dram_parameter("dirT", [128, DIRW], f32, isOutput=False)
    xTp_h = nc.declare_dram_parameter("xTp", [128, XW], f32, isOutput=False)
    out_h = nc.declare_dram_parameter("out", [KSH, B], f32, isOutput=True)

    ctx = contextlib.ExitStack()

    def sb(name, shape, dt=f32):
        return ctx.enter_context(nc.sbuf_tensor(name, shape, dt))[:]

    def ps(name, shape, dt=f32):
        return ctx.enter_context(nc.psum_tensor(name, shape, dt))[:]

    with ctx:
        hrp_t = sb("hrp_t", [128, HRW])
        aux2_t = sb("aux2_t", [128, A2W])
        ident_t = sb("ident_t", [64, 64], bf16)
        v33_t = sb("v33_t", [33, VW33])
        dirT_t = sb("dirT_t", [128, DIRW])
        xTp_t = sb("xTp_t", [128, XW])

        ones_t = sb("ones_t", [128, 3, 512], u16)
        amt_t = sb("amt_t", [128, 3, 512], u16)
        w_t = sb("w_t", [128, 3, 512], u16)
        packed_t = sb("packed_t", [128, 3], u16)
        packhi_t = sb("packhi_t", [64, 3], u16)
        p33u_t = sb("p33u_t", [64, 33], u16)
        p33_t = sb("p33_t", [64, 33], bf16)
        presT_t = sb("presT_t", [33, 64], bf16)
        ohlow_t = sb("ohlow_t", [33, IN], bf16)
        ohhigh_t = sb("ohhigh_t", [33, KSH], bf16)
        rlmax_t = sb("rlmax_t", [128, NT])
        E_t = sb("E_t", [128, NT, KSH], bf16)
        xlT_t = sb("xlT_t", [128, NT, B], bf16)
        z_t = sb("z_t", [KSH, B])
        outT_t = sb("outT_t", [KSH, B])
        warm_t = sb("warm_t", [128, 1])

        presT_ps = ps("presT_ps", [33, 64], bf16)
        plT_ps = ps("plT_ps", [128, NT, B])
        omT_ps = ps("omT_ps", [KSH, B])
        Y_ps = ps("Y_ps", [KSH, B])

        hr_ap = hrp_t[:, :].bitcast(i16)                        # [128, 512]
        rlowpp_ap = aux2_t[:, A_RLPP : A_RLPP + 4].bitcast(bf16)   # [128, 8]
        rhighbb_ap = aux2_t[:, A_RHBB : A_RHBB + 64].bitcast(bf16)  # [128, 128]
        cs_ap = aux2_t[:, A_CS : A_CS + 1]
        cb_ap = aux2_t[:, A_CB : A_CB + 1]
        msk_ap = aux2_t[0:64, A_MSK : A_MSK + 17].bitcast(u16)[:, 0:33]
        vpl_ap = aux2_t[0:33, A_VPL : A_VPL + 1]
        vom_ap = aux2_t[0:33, A_VOM : A_VOM + 1]
        pow_ap = aux2_t[0:33, A_POW : A_POW + 1]
        rl33_ap = v33_t[:, V_RL // 2 : (V_RL + IN) // 2].bitcast(bf16)
        rh33_ap = v33_t[:, V_RH // 2 : (V_RH + KSH) // 2].bitcast(bf16)
        xT_ap = xTp_t[:, :].bitcast(bf16).rearrange("p (t b) -> p t b", t=NT)
        dirT_ap = dirT_t[:, :].bitcast(bf16).rearrange("p (t k) -> p t k", t=NT)

        hr_sem = ctx.enter_context(nc.semaphore("hr_sem"))
        aux_sem = ctx.enter_context(nc.semaphore("aux_sem"))
        dir_sem = ctx.enter_context(nc.semaphore("dir_sem"))
        x_sem = ctx.enter_context(nc.semaphore("x_sem"))
        out_sem = ctx.enter_context(nc.semaphore("out_sem"))
        dve_sem = ctx.enter_context(nc.semaphore("dve_sem"))
        pe_sem = ctx.enter_context(nc.semaphore("pe_sem"))
        act_sem = ctx.enter_context(nc.semaphore("act_sem"))
        block = ctx.enter_context(nc.Block())

        @block.sync
        def _(sync):
            sync.dma_start(out=hrp_t, in_=hrp_h[:, :]).then_inc(hr_sem, 16)
            sync.dma_start(out=aux2_t, in_=aux2_h[:, :]).then_inc(aux_sem, 16)
            sync.wait_ge(out_sem, 16)

        @block.scalar
        def _(scalar):
            scalar.dma_start(out=v33_t, in_=v33_h[:, :]).then_inc(aux_sem, 16)
            scalar.dma_start(out=ident_t, in_=identp_h[:, :]).then_inc(aux_sem, 16)
            scalar.dma_start(out=dirT_t, in_=dirT_h[:, :]).then_inc(dir_sem, 16)
            scalar.dma_start(out=xTp_t, in_=xTp_h[:, :]).then_inc(x_sem, 16)
            # warm the activation table in the DMA shadow (content irrelevant)
            nc.scalar.activation(
                out=warm_t, in_=aux2_t[:, 0:1], func=Act.Identity,
                bias=0.0, scale=1.0,
            )
            scalar.wait_ge(pe_sem, 1)
            nc.scalar.activation(
                out=presT_t, in_=presT_ps, func=Act.Copy, bias=0.0, scale=1.0,
            ).then_inc(act_sem, 1)
            # act=1: presT in SBUF
            scalar.wait_ge(pe_sem, 3)
            nc.scalar.activation(
                out=z_t, in_=Y_ps, func=Act.Identity, bias=cb_ap, scale=cs_ap,
            ).then_inc(act_sem, 1)
            # act=2: z = cs*Y + cb ready
            scalar.wait_ge(dve_sem, 4)
            scalar.dma_start(out=out_h[:, :], in_=outT_t).then_inc(out_sem, 16)

        @block.vector
        def _(vector):
            nc.vector.memset(ones_t, 1)
            vector.drain()
            vector.wait_ge(hr_sem, 16)
            # word1 two-sided clamp emitted first; its second stage hides
            # behind the other amt ops
            nc.vector.tensor_scalar(
                out=amt_t[:, 1, :], in0=hr_ap, scalar1=14, scalar2=0,
                op0=Alu.subtract, op1=Alu.max,
            )
            nc.vector.tensor_scalar(
                out=amt_t[:, 0, :], in0=hr_ap, scalar1=15, scalar2=None,
                op0=Alu.min,
            )
            nc.vector.tensor_scalar(
                out=amt_t[:, 2, :], in0=hr_ap, scalar1=28, scalar2=0,
                op0=Alu.subtract, op1=Alu.max,
            )
            vector.wait_ge(aux_sem, 48)
            nc.vector.tensor_scalar(
                out=rlmax_t, in0=rlowpp_ap, scalar1=1.0, scalar2=None,
                op0=Alu.max,
            )
            vector.drain()
            nc.vector.tensor_scalar(
                out=amt_t[:, 1, :], in0=amt_t[:, 1, :], scalar1=15,
                scalar2=None, op0=Alu.min,
            )
            vector.drain()
            nc.vector.tensor_tensor(
                out=w_t, in0=ones_t, in1=amt_t, op=Alu.logical_shift_left,
            )
            vector.drain()
            nc.vector.tensor_reduce(
                out=packed_t, in_=w_t, axis=mybir.AxisListType.X,
                op=Alu.bitwise_or,
            )
            vector.drain()
            nc.vector.tensor_copy(out=packhi_t, in_=packed_t[64:128, :])
            vector.drain()
            nc.vector.tensor_tensor(
                out=packed_t[0:64, :], in0=packed_t[0:64, :], in1=packhi_t,
                op=Alu.bitwise_or,
            )
            vector.drain()
            # extract 33 presence columns (scaled 2^bit, compensated in the
            # one-hot tables); bitwise ops must stay integer-typed
            nc.vector.tensor_tensor(
                out=p33u_t[:, 0:15],
                in0=packed_t[0:64, 0:1].broadcast_to((64, 15)),
                in1=msk_ap[:, 0:15], op=Alu.bitwise_and,
            )
            nc.vector.tensor_tensor(
                out=p33u_t[:, 15:29],
                in0=packed_t[0:64, 1:2].broadcast_to((64, 14)),
                in1=msk_ap[:, 15:29], op=Alu.bitwise_and,
            )
            nc.vector.tensor_tensor(
                out=p33u_t[:, 29:33],
                in0=packed_t[0:64, 2:3].broadcast_to((64, 4)),
                in1=msk_ap[:, 29:33], op=Alu.bitwise_and,
            )
            vector.drain()
            nc.vector.tensor_copy(out=p33_t, in_=p33u_t).then_inc(dve_sem, 1)
            # dve=1: p33 ready (PE transposes while DVE builds one-hots)
            nc.vector.tensor_scalar(
                out=ohlow_t, in0=rl33_ap, scalar1=vpl_ap, scalar2=pow_ap,
                op0=Alu.is_equal, op1=Alu.mult,
            )
            nc.vector.tensor_scalar(
                out=ohhigh_t, in0=rh33_ap, scalar1=vom_ap, scalar2=pow_ap,
                op0=Alu.is_equal, op1=Alu.mult,
            ).then_inc(dve_sem, 1)
            # dve=2: one-hots ready (PE gathers once presT lands via act=1)
            vector.wait_ge(dir_sem, 16)
            # E[:,t,:] = (r_high >= max(r_low,1)) * dirT  — fused mask+mult
            for t in range(NT):
                nc.vector.scalar_tensor_tensor(
                    out=E_t[:, t, :], in0=rhighbb_ap,
                    scalar=rlmax_t[:, t : t + 1], in1=dirT_ap[:, t, :],
                    op0=Alu.is_ge, op1=Alu.mult,
                )
            vector.wait_ge(x_sem, 16)
            vector.wait_ge(pe_sem, 2)
            nc.vector.tensor_mul(out=xlT_t, in0=xT_ap, in1=plT_ps).then_inc(
                dve_sem, 1
            )
            # dve=3: xlT + (in-order) E ready -> PE main matmul
            vector.wait_ge(act_sem, 2)
            nc.vector.tensor_mul(out=outT_t, in0=omT_ps, in1=z_t).then_inc(
                dve_sem, 1
            )
            # dve=4: outT ready (scalar issues the store)

        @block.tensor
        def _(tensor):
            tensor.wait_ge(aux_sem, 48)  # ident landed
            tensor.wait_ge(dve_sem, 1)
            nc.tensor.transpose(presT_ps, p33_t, ident_t).then_inc(pe_sem, 1)
            # pe=1: presT_ps ready (ACT copies it to SBUF)
            tensor.wait_ge(dve_sem, 2)
            tensor.wait_ge(act_sem, 1)
            for t in range(NT):
                ins = nc.tensor.matmul(
                    plT_ps[:, t, :],
                    ohlow_t[0:33, t * 128 : (t + 1) * 128],
                    presT_t[0:33, :],
                )
            ins.then_inc(pe_sem, 1)
            # pe=2: plT ready
            nc.tensor.matmul(omT_ps, ohhigh_t[0:33, :], presT_t[0:33, :])
            tensor.wait_ge(dve_sem, 3)
            for t in range(NT):
                ins = nc.tensor.matmul(
                    Y_ps, E_t[:, t, :], xlT_t[:, t, :],
                    start=(t == 0), stop=(t == NT - 1),
                )
            ins.then_inc(pe_sem, 1)
            # pe=3: Y ready

    return nc


def _host_tables():
    import ml_dtypes

    ident = np.eye(64, dtype=ml_dtypes.bfloat16)
    bits = np.concatenate(
        [np.arange(15), np.arange(1, 15), np.arange(1, 5)]
    )  # bit index per p33 column
    masks = np.zeros((64, 34), np.uint16)
    masks[:, 0:33] = (np.uint16(1) << bits.astype(np.uint16))[None, :]
    vals = np.concatenate(
        [np.arange(15), np.arange(15, 29), np.arange(29, 33)]
    ).astype(np.float32)  # value per p33 column
    viota_pl = vals[:, None].copy()
    viota_pl[0, 0] = -1.0  # r_low == 0 contributes nothing
    viota_om = vals[:, None].copy()
    powv = (2.0 ** -bits.astype(np.float32))[:, None]
    return ident, masks, viota_pl, viota_om, powv


def _prep_in_maps(inputs):
    """Host-side sharding: layout / dtype transforms only, no arithmetic."""
    import ml_dtypes

    bf = ml_dtypes.bfloat16
    x = np.ascontiguousarray(np.asarray(inputs["x"], dtype=np.float32))
    hr = np.ascontiguousarray(np.asarray(inputs["hidden_rank"], dtype=np.int32))
    r_low = np.asarray(inputs["r_low"], dtype=np.int32)
    r_high = np.asarray(inputs["r_high"], dtype=np.int32)
    direction = np.asarray(inputs["direction"], dtype=np.float32)
    cscale_b = np.asarray(inputs["cscale_b"], dtype=np.float32)
    cbias_b = np.asarray(inputs["cbias_b"], dtype=np.float32)

    # partition p = h*64 + b, free = s: hr2[h*64+b, s] = hr[b, h*512+s]
    hr2 = hr.reshape(B, 2, 512).transpose(1, 0, 2).reshape(128, 512)
    hrp = hr2.astype(np.int16).view(np.float32)  # [128, 256]

    xT3 = x.T.reshape(NT, 128, B).transpose(1, 0, 2)  # [128, NT, B]
    xTp = xT3.reshape(128, -1).astype(bf).view(np.float32)

    rlowf = r_low.astype(np.float32)
    rhighf = r_high.astype(np.float32)
    ident, masks, viota_pl, viota_om, powv = _host_tables()

    aux2 = np.zeros((128, A2W), np.float32)
    aux2[:, A_RLPP : A_RLPP + 4] = (
        np.ascontiguousarray(rlowf.reshape(NT, 128).T).astype(bf).view(np.float32)
    )
    aux2[0:64, A_MSK : A_MSK + 17] = masks.view(np.float32)
    aux2[0:33, A_VPL : A_VPL + 1] = viota_pl
    aux2[0:33, A_VOM : A_VOM + 1] = viota_om
    aux2[0:33, A_POW : A_POW + 1] = powv

    v33 = np.zeros((33, VW33 * 2), bf)
    v33[:, V_RL : V_RL + IN] = rlowf[None, :].astype(bf)

    in_maps = []
    for c in range(NCORES):
        sl = slice(c * KSH, (c + 1) * KSH)
        rh = rhighf[sl]
        aux2c = aux2.copy()
        aux2c[:, A_RHBB : A_RHBB + 64] = (
            np.ascontiguousarray(rh[None, :].repeat(128, 0)).astype(bf).view(np.float32)
        )
        aux2c[:, A_CS] = cscale_b[sl]
        aux2c[:, A_CB] = cbias_b[sl]
        v33c = v33.copy()
        v33c[:, V_RH : V_RH + KSH] = rh[None, :].astype(bf)
        dirT = (
            direction[sl, :].T.reshape(NT, 128, KSH).transpose(1, 0, 2)
            .reshape(128, -1).astype(bf).view(np.float32)
        )
        in_maps.append(
            {
                "hrp": hrp,
                "aux2": aux2c,
                "identp": ident,
                "v33": v33c.view(np.float32),
                "dirT": dirT,
                "xTp": xTp,
            }
        )
    return in_maps


def _run(inputs, trace=False, **kw):
    from concourse.bass_utils import run_bass_kernel_spmd

    if "nc" not in _cached:
        _cached["nc"] = _build_nc()
    nc = _cached["nc"]
    in_maps = _prep_in_maps(inputs)
    res = run_bass_kernel_spmd(
        nc, in_maps, core_ids=list(range(NCORES)), trace=trace, **kw
    )
    out = np.concatenate([np.asarray(r["out"]).T for r in res.results], axis=1)
    return out.astype(np.float32), res


def kernel(**inputs):
    out, _ = _run(inputs, trace=False)
    return out


# revision 13
# speedup vs baseline: 2.3945x; 1.1462x over previous
"""Trainium2 Bass kernel for nn_AMMaskedLinear (v3).

Math: the reference's per-sample weight mask is separable:
    present[b,v] = any_j(hidden_rank[b,j] == v)            (v in 0..32)
    pl[b,i] = present[b, r_low[i]]  & (r_low[i]  != 0)
    om[b,o] = present[b, r_high[o]]
    E[j,k]  = (r_high[k] >= max(r_low[j], 1)) * direction^T[j,k]
    Y[k,b]  = sum_j E[j,k] * pl[j,b] * x[j,b]
    out[k,b] = om[k,b] * (cscale_b[k] * Y[k,b] + cbias_b[k])

Presence bit-pack on [128=(half,b), 512] (3 u16 words, baseline-proven
clamp windows; all clamp-boundary bits are dead):
    w0 = 1 << min(hr, 15)              bits 0..14 <-> values 0..14
    w1 = 1 << clamp(hr-14, 0, 15)      bits 1..14 <-> values 15..28
    w2 = 1 << max(hr-28, 0)            bits 1..4  <-> values 29..32
One tensor_reduce(bitwise_or) along the free axis replaces the baseline's
9-level OR tree.  Extraction is 3 bitwise_ANDs with per-column masks; the
resulting power-of-two scales are compensated by 2^-bit factors baked into
the one-hot gather tables (exact in bf16), so no booleanize pass is needed.

Engine split: DVE owns pack + one-hots + E + xlT + final mul; PE transposes
presence and runs gathers + main matmul; ACT copies presT out of PSUM and
computes z = cs*Y + cb (per-partition scale/bias APs) and issues the bulk
DMAs; Sync issues hr/aux2.  GpSimd is UNUSED on purpose: measured ~27x
slower than DVE per element and it stalls concurrent DVE ops.
"""

import numpy as np

B, IN, OUT, D = 64, 1024, 1024, 32
NCORES = 8
KSH = OUT // NCORES  # 128 outputs per core
NT = IN // 128       # 8 contraction tiles

# aux2 [128, A2W] f32
A_RLPP = 0           # [128, 4]    r_low partition-major bf16 (8 bf16)
A_RHBB = 4           # [128, 64]   r_high shard bcast bf16 (128 bf16)
A_CS = 68            # [128, 1]    cscale_b shard f32 (per-partition k)
A_CB = 69            # [128, 1]    cbias_b shard f32
A_MSK = 70           # [64, 17]    u16 extraction masks (33 used + pad)
A_VPL = 87           # [33, 1]     viota for pl one-hot (v0 col dead)
A_VOM = 88           # [33, 1]     viota for om one-hot
A_POW = 89           # [33, 1]     2^-bit compensation factors
A2W = 90

# v33 [33, VW33] f32 (bf16 payload)
V_RL = 0             # [33, 1024] bf16 r_low bcast
V_RH = 1024          # [33, 128]  bf16 r_high shard bcast
VW33 = 576

HRW = 256            # hrp [128, 256] f32 = [128, 512] i16
DIRW = 512           # dirT [128, 512] f32 = [128, 8, 128] bf16
XW = 256             # xTp  [128, 256] f32 = [128, 8, 64] bf16

_cached = {}


def _build_nc():
    import contextlib

    import concourse.bass as bass
    import concourse.mybir as mybir

    f32 = mybir.dt.float32
    bf16 = mybir.dt.bfloat16
    i16 = mybir.dt.int16
    u16 = mybir.dt.uint16
    Alu = mybir.AluOpType
    Act = mybir.ActivationFunctionType

    nc = bass.Bass()

    hrp_h = nc.declare_dram_parameter("hrp", [128, HRW], f32, isOutput=False)
    aux2_h = nc.declare_dram_parameter("aux2", [128, A2W], f32, isOutput=False)
    identp_h = nc.declare_dram_parameter("identp", [64, 64], bf16, isOutput=False)
    v33_h = nc.declare_dram_parameter("v33", [33, VW33], f32, isOutput=False)
    dirT_h = nc.declare_dram_parameter("dirT", [128, DIRW], f32, isOutput=False)
    xTp_h = nc.declare_dram_parameter("xTp", [128, XW], f32, isOutput=False)
    out_h = nc.declare_dram_parameter("out", [KSH, B], f32, isOutput=True)

    ctx = contextlib.ExitStack()

    def sb(name, shape, dt=f32):
        return ctx.enter_context(nc.sbuf_tensor(name, shape, dt))[:]

    def ps(name, shape, dt=f32):
        return ctx.enter_context(nc.psum_tensor(name, shape, dt))[:]

    with ctx:
        hrp_t = sb("hrp_t", [128, HRW])
        aux2_t = sb("aux2_t", [128, A2W])
        ident_t = sb("ident_t", [64, 64], bf16)
        v33_t = sb("v33_t", [33, VW33])
        dirT_t = sb("dirT_t", [128, DIRW])
        xTp_t = sb("xTp_t", [128, XW])

        ones_t = sb("ones_t", [128, 3, 512], u16)
        amt_t = sb("amt_t", [128, 3, 512], u16)
        w_t = sb("w_t", [128, 3, 512], u16)
        packed_t = sb("packed_t", [128, 3], u16)
        packhi_t = sb("packhi_t", [64, 3], u16)
        p33u_t = sb("p33u_t", [64, 33], u16)
        p33_t = sb("p33_t", [64, 33], bf16)
        presT_t = sb("presT_t", [33, 64], bf16)
        ohlow_t = sb("ohlow_t", [33, IN], bf16)
        ohhigh_t = sb("ohhigh_t", [33, KSH], bf16)
        rlmax_t = sb("rlmax_t", [128, NT], bf16)
        mask_t = sb("mask_t", [128, NT, KSH], bf16)
        E_t = sb("E_t", [128, NT, KSH], bf16)
        xlT_t = sb("xlT_t", [128, NT, B], bf16)
        z_t = sb("z_t", [KSH, B])
        outT_t = sb("outT_t", [KSH, B])
        warm_t = sb("warm_t", [128, 1])

        presT_ps = ps("presT_ps", [33, 64], bf16)
        plT_ps = ps("plT_ps", [128, NT, B])
        omT_ps = ps("omT_ps", [KSH, B])
        Y_ps = ps("Y_ps", [KSH, B])

        hr_ap = hrp_t[:, :].bitcast(i16)                        # [128, 512]
        rlowpp_ap = aux2_t[:, A_RLPP : A_RLPP + 4].bitcast(bf16)   # [128, 8]
        rhighbb_ap = aux2_t[:, A_RHBB : A_RHBB + 64].bitcast(bf16)  # [128, 128]
        cs_ap = aux2_t[:, A_CS : A_CS + 1]
        cb_ap = aux2_t[:, A_CB : A_CB + 1]
        msk_ap = aux2_t[0:64, A_MSK : A_MSK + 17].bitcast(u16)[:, 0:33]
        vpl_ap = aux2_t[0:33, A_VPL : A_VPL + 1]
        vom_ap = aux2_t[0:33, A_VOM : A_VOM + 1]
        pow_ap = aux2_t[0:33, A_POW : A_POW + 1]
        rl33_ap = v33_t[:, V_RL // 2 : (V_RL + IN) // 2].bitcast(bf16)
        rh33_ap = v33_t[:, V_RH // 2 : (V_RH + KSH) // 2].bitcast(bf16)
        xT_ap = xTp_t[:, :].bitcast(bf16).rearrange("p (t b) -> p t b", t=NT)
        dirT_ap = dirT_t[:, :].bitcast(bf16).rearrange("p (t k) -> p t k", t=NT)

        hr_sem = ctx.enter_context(nc.semaphore("hr_sem"))
        aux_sem = ctx.enter_context(nc.semaphore("aux_sem"))
        dir_sem = ctx.enter_context(nc.semaphore("dir_sem"))
        x_sem = ctx.enter_context(nc.semaphore("x_sem"))
        out_sem = ctx.enter_context(nc.semaphore("out_sem"))
        dve_sem = ctx.enter_context(nc.semaphore("dve_sem"))
        pe_sem = ctx.enter_context(nc.semaphore("pe_sem"))
        act_sem = ctx.enter_context(nc.semaphore("act_sem"))
        block = ctx.enter_context(nc.Block())

        @block.sync
        def _(sync):
            # ALL dmas on sync: issuing bulk DMAs from the ACT engine adds a
            # multi-us DGE-drain at block exit (measured v3); sync's doesn't
            sync.dma_start(out=hrp_t, in_=hrp_h[:, :]).then_inc(hr_sem, 16)
            sync.dma_start(out=aux2_t, in_=aux2_h[:, :]).then_inc(aux_sem, 16)
            sync.dma_start(out=v33_t, in_=v33_h[:, :]).then_inc(aux_sem, 16)
            sync.dma_start(out=ident_t, in_=identp_h[:, :]).then_inc(aux_sem, 16)
            sync.dma_start(out=dirT_t, in_=dirT_h[:, :]).then_inc(dir_sem, 16)
            sync.dma_start(out=xTp_t, in_=xTp_h[:, :]).then_inc(x_sem, 16)
            sync.wait_ge(dve_sem, 4)
            sync.dma_start(out=out_h[:, :], in_=outT_t).then_inc(out_sem, 16)
            sync.wait_ge(out_sem, 16)

        @block.scalar
        def _(scalar):
            # warm the activation table in the DMA shadow (content irrelevant)
            nc.scalar.activation(
                out=warm_t, in_=aux2_t[:, 0:1], func=Act.Identity,
                bias=0.0, scale=1.0,
            )
            scalar.wait_ge(pe_sem, 1)
            nc.scalar.activation(
                out=presT_t, in_=presT_ps, func=Act.Copy, bias=0.0, scale=1.0,
            ).then_inc(act_sem, 1)
            # act=1: presT in SBUF

        @block.vector
        def _(vector):
            nc.vector.memset(ones_t, 1)
            vector.drain()
            vector.wait_ge(hr_sem, 16)
            # word1 two-sided clamp emitted first; its second stage hides
            # behind the other amt ops
            nc.vector.tensor_scalar(
                out=amt_t[:, 1, :], in0=hr_ap, scalar1=14, scalar2=0,
                op0=Alu.subtract, op1=Alu.max,
            )
            nc.vector.tensor_scalar(
                out=amt_t[:, 0, :], in0=hr_ap, scalar1=15, scalar2=None,
                op0=Alu.min,
            )
            nc.vector.tensor_scalar(
                out=amt_t[:, 2, :], in0=hr_ap, scalar1=28, scalar2=0,
                op0=Alu.subtract, op1=Alu.max,
            )
            vector.wait_ge(aux_sem, 48)
            nc.vector.tensor_scalar(
                out=rlmax_t, in0=rlowpp_ap, scalar1=1.0, scalar2=None,
                op0=Alu.max,
            )
            vector.drain()
            nc.vector.tensor_scalar(
                out=amt_t[:, 1, :], in0=amt_t[:, 1, :], scalar1=15,
                scalar2=None, op0=Alu.min,
            )
            vector.drain()
            nc.vector.tensor_tensor(
                out=w_t, in0=ones_t, in1=amt_t, op=Alu.logical_shift_left,
            )
            vector.drain()
            nc.vector.tensor_tensor(
                out=w_t[:, :, 0:256], in0=w_t[:, :, 0:256],
                in1=w_t[:, :, 256:512], op=Alu.bitwise_or,
            )
            vector.drain()
            nc.vector.tensor_reduce(
                out=packed_t, in_=w_t[:, :, 0:256], axis=mybir.AxisListType.X,
                op=Alu.bitwise_or,
            )
            vector.drain()
            nc.vector.tensor_copy(out=packhi_t, in_=packed_t[64:128, :])
            vector.drain()
            nc.vector.tensor_tensor(
                out=packed_t[0:64, :], in0=packed_t[0:64, :], in1=packhi_t,
                op=Alu.bitwise_or,
            )
            vector.drain()
            # extract 33 presence columns (scaled 2^bit, compensated in the
            # one-hot tables); bitwise ops must stay integer-typed
            nc.vector.tensor_tensor(
                out=p33u_t[:, 0:15],
                in0=packed_t[0:64, 0:1].broadcast_to((64, 15)),
                in1=msk_ap[:, 0:15], op=Alu.bitwise_and,
            )
            nc.vector.tensor_tensor(
                out=p33u_t[:, 15:29],
                in0=packed_t[0:64, 1:2].broadcast_to((64, 14)),
                in1=msk_ap[:, 15:29], op=Alu.bitwise_and,
            )
            nc.vector.tensor_tensor(
                out=p33u_t[:, 29:33],
                in0=packed_t[0:64, 2:3].broadcast_to((64, 4)),
                in1=msk_ap[:, 29:33], op=Alu.bitwise_and,
            )
            vector.drain()
            nc.vector.tensor_copy(out=p33_t, in_=p33u_t).then_inc(dve_sem, 1)
            # dve=1: p33 ready (PE transposes while DVE builds one-hots)
            nc.vector.tensor_scalar(
                out=ohlow_t, in0=rl33_ap, scalar1=vpl_ap, scalar2=pow_ap,
                op0=Alu.is_equal, op1=Alu.mult,
            )
            nc.vector.tensor_scalar(
                out=ohhigh_t, in0=rh33_ap, scalar1=vom_ap, scalar2=pow_ap,
                op0=Alu.is_equal, op1=Alu.mult,
            ).then_inc(dve_sem, 1)
            # dve=2: one-hots ready (PE gathers once presT lands via act=1)
            vector.wait_ge(dir_sem, 16)
            # mask[j,t,k] = (max(r_low[t*128+j],1) <= r_high[k]); one big TT
            # (per-op overhead dominates small DVE ops — 2 ops beat 8)
            nc.vector.tensor_tensor(
                out=mask_t,
                in0=rlmax_t[:, :, None].broadcast_to((128, NT, KSH)),
                in1=rhighbb_ap[:, None, :].broadcast_to((128, NT, KSH)),
                op=Alu.is_le,
            )
            vector.drain()
            nc.vector.tensor_mul(out=E_t, in0=mask_t, in1=dirT_ap)
            vector.wait_ge(x_sem, 16)
            vector.wait_ge(pe_sem, 2)
            nc.vector.tensor_mul(out=xlT_t, in0=xT_ap, in1=plT_ps).then_inc(
                dve_sem, 1
            )
            # dve=3: xlT + (in-order) E ready -> PE main matmul
            vector.wait_ge(pe_sem, 3)
            nc.vector.scalar_tensor_tensor(
                out=z_t, in0=Y_ps, scalar=cs_ap,
                in1=cb_ap.broadcast_to((KSH, B)),
                op0=Alu.mult, op1=Alu.add,
            )
            vector.drain()
            nc.vector.tensor_mul(out=outT_t, in0=omT_ps, in1=z_t).then_inc(
                dve_sem, 1
            )
            # dve=4: outT ready (sync issues the store)

        @block.tensor
        def _(tensor):
            tensor.wait_ge(aux_sem, 48)  # ident landed
            tensor.wait_ge(dve_sem, 1)
            nc.tensor.transpose(presT_ps, p33_t, ident_t).then_inc(pe_sem, 1)
            # pe=1: presT_ps ready (ACT copies it to SBUF)
            tensor.wait_ge(dve_sem, 2)
            tensor.wait_ge(act_sem, 1)
            for t in range(NT):
                ins = nc.tensor.matmul(
                    plT_ps[:, t, :],
                    ohlow_t[0:33, t * 128 : (t + 1) * 128],
                    presT_t[0:33, :],
                )
            ins.then_inc(pe_sem, 1)
            # pe=2: plT ready
            nc.tensor.matmul(omT_ps, ohhigh_t[0:33, :], presT_t[0:33, :])
            tensor.wait_ge(dve_sem, 3)
            for t in range(NT):
                ins = nc.tensor.matmul(
                    Y_ps, E_t[:, t, :], xlT_t[:, t, :],
                    start=(t == 0), stop=(t == NT - 1),
                )
            ins.then_inc(pe_sem, 1)
            # pe=3: Y ready

    return nc


def _host_tables():
    import ml_dtypes

    ident = np.eye(64, dtype=ml_dtypes.bfloat16)
    bits = np.concatenate(
        [np.arange(15), np.arange(1, 15), np.arange(1, 5)]
    )  # bit index per p33 column
    masks = np.zeros((64, 34), np.uint16)
    masks[:, 0:33] = (np.uint16(1) << bits.astype(np.uint16))[None, :]
    vals = np.concatenate(
        [np.arange(15), np.arange(15, 29), np.arange(29, 33)]
    ).astype(np.float32)  # value per p33 column
    viota_pl = vals[:, None].copy()
    viota_pl[0, 0] = -1.0  # r_low == 0 contributes nothing
    viota_om = vals[:, None].copy()
    powv = (2.0 ** -bits.astype(np.float32))[:, None]
    return ident, masks, viota_pl, viota_om, powv


def _prep_in_maps(inputs):
    """Host-side sharding: layout / dtype transforms only, no arithmetic."""
    import ml_dtypes

    bf = ml_dtypes.bfloat16
    x = np.ascontiguousarray(np.asarray(inputs["x"], dtype=np.float32))
    hr = np.ascontiguousarray(np.asarray(inputs["hidden_rank"], dtype=np.int32))
    r_low = np.asarray(inputs["r_low"], dtype=np.int32)
    r_high = np.asarray(inputs["r_high"], dtype=np.int32)
    direction = np.asarray(inputs["direction"], dtype=np.float32)
    cscale_b = np.asarray(inputs["cscale_b"], dtype=np.float32)
    cbias_b = np.asarray(inputs["cbias_b"], dtype=np.float32)

    # partition p = h*64 + b, free = s: hr2[h*64+b, s] = hr[b, h*512+s]
    hr2 = hr.reshape(B, 2, 512).transpose(1, 0, 2).reshape(128, 512)
    hrp = hr2.astype(np.int16).view(np.float32)  # [128, 256]

    xT3 = x.T.reshape(NT, 128, B).transpose(1, 0, 2)  # [128, NT, B]
    xTp = xT3.reshape(128, -1).astype(bf).view(np.float32)

    rlowf = r_low.astype(np.float32)
    rhighf = r_high.astype(np.float32)
    ident, masks, viota_pl, viota_om, powv = _host_tables()

    aux2 = np.zeros((128, A2W), np.float32)
    aux2[:, A_RLPP : A_RLPP + 4] = (
        np.ascontiguousarray(rlowf.reshape(NT, 128).T).astype(bf).view(np.float32)
    )
    aux2[0:64, A_MSK : A_MSK + 17] = masks.view(np.float32)
    aux2[0:33, A_VPL : A_VPL + 1] = viota_pl
    aux2[0:33, A_VOM : A_VOM + 1] = viota_om
    aux2[0:33, A_POW : A_POW + 1] = powv

    v33 = np.zeros((33, VW33 * 2), bf)
    v33[:, V_RL : V_RL + IN] = rlowf[None, :].astype(bf)

    in_maps = []
    for c in range(NCORES):
        sl = slice(c * KSH, (c + 1) * KSH)
        rh = rhighf[sl]
        aux2c = aux2.copy()
        aux2c[:, A_RHBB : A_RHBB + 64] = (
            np.ascontiguousarray(rh[None, :].repeat(128, 0)).astype(bf).view(np.float32)
        )
        aux2c[:, A_CS] = cscale_b[sl]
        aux2c[:, A_CB] = cbias_b[sl]
        v33c = v33.copy()
        v33c[:, V_RH : V_RH + KSH] = rh[None, :].astype(bf)
        dirT = (
            direction[sl, :].T.reshape(NT, 128, KSH).transpose(1, 0, 2)
            .reshape(128, -1).astype(bf).view(np.float32)
        )
        in_maps.append(
            {
                "hrp": hrp,
                "aux2": aux2c,
                "identp": ident,
                "v33": v33c.view(np.float32),
                "dirT": dirT,
                "xTp": xTp,
            }
        )
    return in_maps


def _run(inputs, trace=False, **kw):
    from concourse.bass_utils import run_bass_kernel_spmd

    if "nc" not in _cached:
        _cached["nc"] = _build_nc()
    nc = _cached["nc"]
    in_maps = _prep_in_maps(inputs)
    res = run_bass_kernel_spmd(
        nc, in_maps, core_ids=list(range(NCORES)), trace=trace, **kw
    )
    out = np.concatenate([np.asarray(r["out"]).T for r in res.results], axis=1)
    return out.astype(np.float32), res


def kernel(**inputs):
    out, _ = _run(inputs, trace=False)
    return out


# revision 15
# speedup vs baseline: 2.4722x; 1.0324x over previous
"""Trainium2 Bass kernel for nn_AMMaskedLinear (v3).

Math: the reference's per-sample weight mask is separable:
    present[b,v] = any_j(hidden_rank[b,j] == v)            (v in 0..32)
    pl[b,i] = present[b, r_low[i]]  & (r_low[i]  != 0)
    om[b,o] = present[b, r_high[o]]
    E[j,k]  = (r_high[k] >= max(r_low[j], 1)) * direction^T[j,k]
    Y[k,b]  = sum_j E[j,k] * pl[j,b] * x[j,b]
    out[k,b] = om[k,b] * (cscale_b[k] * Y[k,b] + cbias_b[k])

Presence bit-pack on [128=(half,b), 512] (3 u16 words, baseline-proven
clamp windows; all clamp-boundary bits are dead):
    w0 = 1 << min(hr, 15)              bits 0..14 <-> values 0..14
    w1 = 1 << clamp(hr-14, 0, 15)      bits 1..14 <-> values 15..28
    w2 = 1 << max(hr-28, 0)            bits 1..4  <-> values 29..32
One tensor_reduce(bitwise_or) along the free axis replaces the baseline's
9-level OR tree.  Extraction is 3 bitwise_ANDs with per-column masks; the
resulting power-of-two scales are compensated by 2^-bit factors baked into
the one-hot gather tables (exact in bf16), so no booleanize pass is needed.

Engine split: DVE owns pack + one-hots + E + xlT + final mul; PE transposes
presence and runs gathers + main matmul; ACT copies presT out of PSUM and
computes z = cs*Y + cb (per-partition scale/bias APs) and issues the bulk
DMAs; Sync issues hr/aux2.  GpSimd is UNUSED on purpose: measured ~27x
slower than DVE per element and it stalls concurrent DVE ops.
"""

import numpy as np

B, IN, OUT, D = 64, 1024, 1024, 32
NCORES = 8
KSH = OUT // NCORES  # 128 outputs per core
NT = IN // 128       # 8 contraction tiles

# aux2 [128, A2W] f32
A_RLPP = 0           # [128, 4]    r_low partition-major bf16 (8 bf16)
A_RHBB = 4           # [128, 64]   r_high shard bcast bf16 (128 bf16)
A_CS = 68            # [128, 1]    cscale_b shard f32 (per-partition k)
A_CB = 69            # [128, 1]    cbias_b shard f32
A_MSK = 70           # [64, 17]    u16 extraction masks (33 used + pad)
A_VPL = 87           # [33, 1]     viota for pl one-hot (v0 col dead)
A_VOM = 88           # [33, 1]     viota for om one-hot
A_POW = 89           # [33, 1]     2^-bit compensation factors
A2W = 90

# v33 [33, VW33] f32 (bf16 payload)
V_RL = 0             # [33, 1024] bf16 r_low bcast
V_RH = 1024          # [33, 128]  bf16 r_high shard bcast
VW33 = 576

HRW = 256            # hrp [128, 256] f32 = [128, 512] i16
DIRW = 512           # dirT [128, 512] f32 = [128, 8, 128] bf16
XW = 256             # xTp  [128, 256] f32 = [128, 8, 64] bf16

_cached = {}


def _build_nc():
    import contextlib

    import concourse.bass as bass
    import concourse.mybir as mybir

    f32 = mybir.dt.float32
    bf16 = mybir.dt.bfloat16
    i16 = mybir.dt.int16
    u16 = mybir.dt.uint16
    Alu = mybir.AluOpType
    Act = mybir.ActivationFunctionType

    nc = bass.Bass()

    hrp_h = nc.declare_dram_parameter("hrp", [128, HRW], f32, isOutput=False)
    aux2_h = nc.declare_dram_parameter("aux2", [128, A2W], f32, isOutput=False)
    identp_h = nc.declare_dram_parameter("identp", [64, 64], bf16, isOutput=False)
    v33_h = nc.declare_dram_parameter("v33", [33, VW33], f32, isOutput=False)
    dirT_h = nc.declare_dram_parameter("dirT", [128, DIRW], f32, isOutput=False)
    xTp_h = nc.declare_dram_parameter("xTp", [128, XW], f32, isOutput=False)
    out_h = nc.declare_dram_parameter("out", [KSH, B], f32, isOutput=True)

    ctx = contextlib.ExitStack()

    def sb(name, shape, dt=f32):
        return ctx.enter_context(nc.sbuf_tensor(name, shape, dt))[:]

    def ps(name, shape, dt=f32):
        return ctx.enter_context(nc.psum_tensor(name, shape, dt))[:]

    with ctx:
        hrp_t = sb("hrp_t", [128, HRW])
        aux2_t = sb("aux2_t", [128, A2W])
        ident_t = sb("ident_t", [64, 64], bf16)
        v33_t = sb("v33_t", [33, VW33])
        dirT_t = sb("dirT_t", [128, DIRW])
        xTp_t = sb("xTp_t", [128, XW])

        ones_t = sb("ones_t", [128, 3, 512], u16)
        amt_t = sb("amt_t", [128, 3, 512], u16)
        w_t = sb("w_t", [128, 3, 512], u16)
        packed_t = sb("packed_t", [128, 3], u16)
        packhi_t = sb("packhi_t", [64, 3], u16)
        p33u_t = sb("p33u_t", [64, 33], u16)
        p33_t = sb("p33_t", [64, 33], bf16)
        presT_t = sb("presT_t", [33, 64], bf16)
        oh_t = sb("oh_t", [33, IN + KSH], bf16)
        rlmax_t = sb("rlmax_t", [128, NT], bf16)
        mask_t = sb("mask_t", [128, NT, KSH], bf16)
        E_t = sb("E_t", [128, NT, KSH], bf16)
        xlT_t = sb("xlT_t", [128, NT, B], bf16)
        z_t = sb("z_t", [KSH, B])
        outT_t = sb("outT_t", [KSH, B])
        warm_t = sb("warm_t", [128, 1])

        presT_ps = ps("presT_ps", [33, 64], bf16)
        plT_ps = ps("plT_ps", [128, NT, B])
        omT_ps = ps("omT_ps", [KSH, B])
        Y_ps = ps("Y_ps", [KSH, B])

        hr_ap = hrp_t[:, :].bitcast(i16)                        # [128, 512]
        rlowpp_ap = aux2_t[:, A_RLPP : A_RLPP + 4].bitcast(bf16)   # [128, 8]
        rhighbb_ap = aux2_t[:, A_RHBB : A_RHBB + 64].bitcast(bf16)  # [128, 128]
        cs_ap = aux2_t[:, A_CS : A_CS + 1]
        cb_ap = aux2_t[:, A_CB : A_CB + 1]
        msk_ap = aux2_t[0:64, A_MSK : A_MSK + 17].bitcast(u16)[:, 0:33]
        vpl_ap = aux2_t[0:33, A_VPL : A_VPL + 1]
        vom_ap = aux2_t[0:33, A_VOM : A_VOM + 1]
        pow_ap = aux2_t[0:33, A_POW : A_POW + 1]
        vv_ap = v33_t[:, 0 : (IN + KSH) // 2].bitcast(bf16)
        xT_ap = xTp_t[:, :].bitcast(bf16).rearrange("p (t b) -> p t b", t=NT)
        dirT_ap = dirT_t[:, :].bitcast(bf16).rearrange("p (t k) -> p t k", t=NT)

        hr_sem = ctx.enter_context(nc.semaphore("hr_sem"))
        aux_sem = ctx.enter_context(nc.semaphore("aux_sem"))
        dir_sem = ctx.enter_context(nc.semaphore("dir_sem"))
        x_sem = ctx.enter_context(nc.semaphore("x_sem"))
        out_sem = ctx.enter_context(nc.semaphore("out_sem"))
        dve_sem = ctx.enter_context(nc.semaphore("dve_sem"))
        pe_sem = ctx.enter_context(nc.semaphore("pe_sem"))
        act_sem = ctx.enter_context(nc.semaphore("act_sem"))
        block = ctx.enter_context(nc.Block())

        @block.sync
        def _(sync):
            # ALL dmas on sync: issuing bulk DMAs from the ACT engine adds a
            # multi-us DGE-drain at block exit (measured v3); sync's doesn't
            sync.dma_start(out=hrp_t, in_=hrp_h[:, :]).then_inc(hr_sem, 16)
            sync.dma_start(out=aux2_t, in_=aux2_h[:, :]).then_inc(aux_sem, 16)
            sync.dma_start(out=v33_t, in_=v33_h[:, :]).then_inc(aux_sem, 16)
            sync.dma_start(out=dirT_t, in_=dirT_h[:, :]).then_inc(dir_sem, 16)
            sync.dma_start(out=xTp_t, in_=xTp_h[:, :]).then_inc(x_sem, 16)
            sync.dma_start(out=ident_t, in_=identp_h[:, :]).then_inc(aux_sem, 16)
            sync.wait_ge(dve_sem, 4)
            sync.dma_start(out=out_h[:, :], in_=outT_t).then_inc(out_sem, 16)
            sync.wait_ge(out_sem, 16)

        @block.scalar
        def _(scalar):
            # warm the activation table in the DMA shadow (content irrelevant)
            nc.scalar.activation(
                out=warm_t, in_=aux2_t[:, 0:1], func=Act.Identity,
                bias=0.0, scale=1.0,
            )
            scalar.wait_ge(pe_sem, 1)
            nc.scalar.activation(
                out=presT_t, in_=presT_ps, func=Act.Copy, bias=0.0, scale=1.0,
            ).then_inc(act_sem, 1)
            # act=1: presT in SBUF

        @block.vector
        def _(vector):
            nc.vector.memset(ones_t, 1)
            vector.drain()
            vector.wait_ge(hr_sem, 16)
            # word1 two-sided clamp emitted first; its second stage hides
            # behind the other amt ops
            nc.vector.tensor_scalar(
                out=amt_t[:, 1, :], in0=hr_ap, scalar1=14, scalar2=0,
                op0=Alu.subtract, op1=Alu.max,
            )
            nc.vector.tensor_scalar(
                out=amt_t[:, 0, :], in0=hr_ap, scalar1=15, scalar2=None,
                op0=Alu.min,
            )
            nc.vector.tensor_scalar(
                out=amt_t[:, 2, :], in0=hr_ap, scalar1=28, scalar2=0,
                op0=Alu.subtract, op1=Alu.max,
            )
            vector.drain()
            nc.vector.tensor_scalar(
                out=amt_t[:, 1, :], in0=amt_t[:, 1, :], scalar1=15,
                scalar2=None, op0=Alu.min,
            )
            vector.drain()
            nc.vector.tensor_tensor(
                out=w_t, in0=ones_t, in1=amt_t, op=Alu.logical_shift_left,
            )
            vector.drain()
            nc.vector.tensor_tensor(
                out=w_t[:, :, 0:256], in0=w_t[:, :, 0:256],
                in1=w_t[:, :, 256:512], op=Alu.bitwise_or,
            )
            vector.drain()
            nc.vector.tensor_reduce(
                out=packed_t, in_=w_t[:, :, 0:256], axis=mybir.AxisListType.X,
                op=Alu.bitwise_or,
            )
            vector.drain()
            nc.vector.tensor_copy(out=packhi_t, in_=packed_t[64:128, :])
            vector.drain()
            nc.vector.tensor_tensor(
                out=packed_t[0:64, :], in0=packed_t[0:64, :], in1=packhi_t,
                op=Alu.bitwise_or,
            )
            vector.drain()
            # extract 33 presence columns (scaled 2^bit, compensated in the
            # one-hot tables); bitwise ops must stay integer-typed
            nc.vector.tensor_tensor(
                out=p33u_t[:, 0:14],
                in0=packed_t[0:64, 0:1].broadcast_to((64, 14)),
                in1=msk_ap[:, 0:14], op=Alu.bitwise_and,
            )
            nc.vector.tensor_tensor(
                out=p33u_t[:, 14:28],
                in0=packed_t[0:64, 1:2].broadcast_to((64, 14)),
                in1=msk_ap[:, 14:28], op=Alu.bitwise_and,
            )
            nc.vector.tensor_tensor(
                out=p33u_t[:, 28:32],
                in0=packed_t[0:64, 2:3].broadcast_to((64, 4)),
                in1=msk_ap[:, 28:32], op=Alu.bitwise_and,
            )
            nc.vector.tensor_tensor(
                out=p33u_t[:, 32:33], in0=packed_t[0:64, 0:1],
                in1=msk_ap[:, 32:33], op=Alu.bitwise_and,
            )
            vector.drain()
            nc.vector.tensor_copy(out=p33_t, in_=p33u_t).then_inc(dve_sem, 1)
            # dve=1: p33 ready (PE transposes while DVE builds the one-hots)
            vector.wait_ge(aux_sem, 32)
            # single is_eq*pow over r_low||r_high; the pl gather implements
            # (r_low != 0) by skipping the value-0 partition row instead of a
            # separate viota
            nc.vector.tensor_scalar(
                out=oh_t, in0=vv_ap, scalar1=vom_ap, scalar2=pow_ap,
                op0=Alu.is_equal, op1=Alu.mult,
            ).then_inc(dve_sem, 1)
            # dve=2: one-hots ready (PE gathers once presT lands via act=1)
            nc.vector.tensor_scalar(
                out=rlmax_t, in0=rlowpp_ap, scalar1=1.0, scalar2=None,
                op0=Alu.max,
            )
            vector.drain()
            vector.wait_ge(dir_sem, 16)
            # mask[j,t,k] = (max(r_low[t*128+j],1) <= r_high[k]); one big TT
            # (per-op overhead dominates small DVE ops — 2 ops beat 8)
            nc.vector.tensor_tensor(
                out=mask_t,
                in0=rlmax_t[:, :, None].broadcast_to((128, NT, KSH)),
                in1=rhighbb_ap[:, None, :].broadcast_to((128, NT, KSH)),
                op=Alu.is_le,
            )
            vector.drain()
            nc.vector.tensor_mul(out=E_t, in0=mask_t, in1=dirT_ap)
            vector.wait_ge(x_sem, 16)
            vector.wait_ge(pe_sem, 2)
            nc.vector.tensor_mul(out=xlT_t, in0=xT_ap, in1=plT_ps).then_inc(
                dve_sem, 1
            )
            # dve=3: xlT + (in-order) E ready -> PE main matmul
            vector.wait_ge(pe_sem, 3)
            nc.vector.scalar_tensor_tensor(
                out=z_t, in0=Y_ps, scalar=cs_ap,
                in1=cb_ap.broadcast_to((KSH, B)),
                op0=Alu.mult, op1=Alu.add,
            )
            vector.drain()
            nc.vector.tensor_mul(out=outT_t, in0=omT_ps, in1=z_t).then_inc(
                dve_sem, 1
            )
            # dve=4: outT ready (sync issues the store)

        @block.tensor
        def _(tensor):
            tensor.wait_ge(aux_sem, 48)  # ident landed
            tensor.wait_ge(dve_sem, 1)
            nc.tensor.transpose(presT_ps, p33_t, ident_t).then_inc(pe_sem, 1)
            # pe=1: presT_ps ready (ACT copies it to SBUF)
            tensor.wait_ge(dve_sem, 2)
            tensor.wait_ge(act_sem, 1)
            for t in range(NT):
                ins = nc.tensor.matmul(
                    plT_ps[:, t, :],
                    oh_t[0:32, t * 128 : (t + 1) * 128],
                    presT_t[0:32, :],
                )
            ins.then_inc(pe_sem, 1)
            # pe=2: plT ready
            nc.tensor.matmul(omT_ps, oh_t[0:33, IN : IN + KSH], presT_t[0:33, :])
            tensor.wait_ge(dve_sem, 3)
            for t in range(NT):
                ins = nc.tensor.matmul(
                    Y_ps, E_t[:, t, :], xlT_t[:, t, :],
                    start=(t == 0), stop=(t == NT - 1),
                )
            ins.then_inc(pe_sem, 1)
            # pe=3: Y ready

    return nc


def _host_tables():
    import ml_dtypes

    ident = np.eye(64, dtype=ml_dtypes.bfloat16)
    # p33 column -> (word bit, value); value 0 sits at column 32 so the pl
    # gather can drop it by contracting partitions 0:32 only
    bits = np.concatenate(
        [np.arange(1, 15), np.arange(1, 15), np.arange(1, 5), [0]]
    )
    masks = np.zeros((64, 34), np.uint16)
    masks[:, 0:33] = (np.uint16(1) << bits.astype(np.uint16))[None, :]
    vals = np.concatenate(
        [np.arange(1, 15), np.arange(15, 29), np.arange(29, 33), [0]]
    ).astype(np.float32)
    viota_pl = vals[:, None].copy()
    viota_om = vals[:, None].copy()
    powv = (2.0 ** -bits.astype(np.float32))[:, None]
    return ident, masks, viota_pl, viota_om, powv


def _prep_in_maps(inputs):
    """Host-side sharding: layout / dtype transforms only, no arithmetic."""
    import ml_dtypes

    bf = ml_dtypes.bfloat16
    x = np.ascontiguousarray(np.asarray(inputs["x"], dtype=np.float32))
    hr = np.ascontiguousarray(np.asarray(inputs["hidden_rank"], dtype=np.int32))
    r_low = np.asarray(inputs["r_low"], dtype=np.int32)
    r_high = np.asarray(inputs["r_high"], dtype=np.int32)
    direction = np.asarray(inputs["direction"], dtype=np.float32)
    cscale_b = np.asarray(inputs["cscale_b"], dtype=np.float32)
    cbias_b = np.asarray(inputs["cbias_b"], dtype=np.float32)

    # partition p = h*64 + b, free = s: hr2[h*64+b, s] = hr[b, h*512+s]
    hr2 = hr.reshape(B, 2, 512).transpose(1, 0, 2).reshape(128, 512)
    hrp = hr2.astype(np.int16).view(np.float32)  # [128, 256]

    xT3 = x.T.reshape(NT, 128, B).transpose(1, 0, 2)  # [128, NT, B]
    xTp = xT3.reshape(128, -1).astype(bf).view(np.float32)

    rlowf = r_low.astype(np.float32)
    rhighf = r_high.astype(np.float32)
    ident, masks, viota_pl, viota_om, powv = _host_tables()

    aux2 = np.zeros((128, A2W), np.float32)
    aux2[:, A_RLPP : A_RLPP + 4] = (
        np.ascontiguousarray(rlowf.reshape(NT, 128).T).astype(bf).view(np.float32)
    )
    aux2[0:64, A_MSK : A_MSK + 17] = masks.view(np.float32)
    aux2[0:33, A_VPL : A_VPL + 1] = viota_pl
    aux2[0:33, A_VOM : A_VOM + 1] = viota_om
    aux2[0:33, A_POW : A_POW + 1] = powv

    v33 = np.zeros((33, VW33 * 2), bf)
    v33[:, V_RL : V_RL + IN] = rlowf[None, :].astype(bf)

    in_maps = []
    for c in range(NCORES):
        sl = slice(c * KSH, (c + 1) * KSH)
        rh = rhighf[sl]
        aux2c = aux2.copy()
        aux2c[:, A_RHBB : A_RHBB + 64] = (
            np.ascontiguousarray(rh[None, :].repeat(128, 0)).astype(bf).view(np.float32)
        )
        aux2c[:, A_CS] = cscale_b[sl]
        aux2c[:, A_CB] = cbias_b[sl]
        v33c = v33.copy()
        v33c[:, V_RH : V_RH + KSH] = rh[None, :].astype(bf)
        dirT = (
            direction[sl, :].T.reshape(NT, 128, KSH).transpose(1, 0, 2)
            .reshape(128, -1).astype(bf).view(np.float32)
        )
        in_maps.append(
            {
                "hrp": hrp,
                "aux2": aux2c,
                "identp": ident,
                "v33": v33c.view(np.float32),
                "dirT": dirT,
                "xTp": xTp,
            }
        )
    return in_maps


def _run(inputs, trace=False, **kw):
    from concourse.bass_utils import run_bass_kernel_spmd

    if "nc" not in _cached:
        _cached["nc"] = _build_nc()
    nc = _cached["nc"]
    in_maps = _prep_in_maps(inputs)
    res = run_bass_kernel_spmd(
        nc, in_maps, core_ids=list(range(NCORES)), trace=trace, **kw
    )
    out = np.concatenate([np.asarray(r["out"]).T for r in res.results], axis=1)
    return out.astype(np.float32), res


def kernel(**inputs):
    out, _ = _run(inputs, trace=False)
    return out
